# revision 38
# baseline (speedup 1.0000x reference)
"""Causal self-attention on 8 Trainium2 NeuronCores — v6.2.

Sharding: batch (2) x head-groups (4), 3 heads per core; per-core partial
out-projections summed on the host (tensor-parallel all-reduce).

Design (vs v5 baseline, ~1.4x faster in interleaved A/B):
- All matmul operands bf16 (fp32 PSUM accumulation); DMA traffic halved,
  y written as bf16 partials and summed on host in fp32.
- x is transposed on the host: xT [768, t] is DMA'd straight into the
  resident SBUF tiles the projections consume — no PE transposes or DVE
  copies for x.
- S = K^T Q matmuls are K=64; pairs of them run concurrently on disjoint
  PE row-groups via tile_position (h0 rows 0-63 || h1 rows 64-127; h2 is
  paired with itself via duplicated Wq2/Wk2 projection columns so its
  kbi-pairs land on both row halves).
- Causal handling ("pv" mode): S and exp run full-width (partial-width S
  rects crash the device at t>=1536), PV matmuls read only [off:512] of
  each diagonal rect, and only the leading 128x128 triangle of each
  diagonal block is masked on gpsimd.
- Projections are emitted in pairs of 512-row blocks so each weight
  chunk's LDWEIGHTS serves two matmuls; out-proj loops are h-major for
  the same reason. Extras are drained at qb boundaries only when their
  deadline requires it.
- Attention core otherwise follows v5: exp on ACT (scale=1/8), softmax
  denominator via an extra all-ones column in the V tiles, reciprocal +
  PE-broadcast + DVE multiply for normalization, head-stacked out-proj
  accumulated in PSUM.
(fp8-DoubleRow QK projections were implemented behind K_PROJ_DR but
measured slower on HW — extra x DMA + DR ldweights penalty; off.)
"""

import contextlib

import numpy as np
import ml_dtypes

import concourse.bass as bass
import concourse.mybir as mybir
from concourse import bacc
from concourse import tile
from concourse.bass_utils import run_bass_kernel_spmd
from concourse.masks import make_identity

F32 = mybir.dt.float32
F32R = mybir.dt.float32r
BF16 = mybir.dt.bfloat16

EMBED = 768
NHEAD = 12
DH = 64
B = 2
T = 4096
HPC = 3
NCORES = 8
QW = 512
WCOLS = 704  # 5.5 mc blocks: q01 k01 q22 k22 v01 v2

Act = mybir.ActivationFunctionType

import os
USE_TILE_POSITION = os.environ.get("K_NO_TILEPOS", "") == ""
BF16_VT = os.environ.get("K_F32_VT", "") == ""
EAGER_X = os.environ.get("K_EAGER_X", "") != ""
# K_TRIM: "pv" (default) = full-width S/exp, trimmed PV matmuls;
# "all" = also trim S/exp rects (crashes HW at t>=1536 — cause unknown);
# "none" = full-width everything
TRIM = os.environ.get("K_TRIM", "pv")
if os.environ.get("K_NO_TRIM", "") != "":
    TRIM = "none"
NO_TRIM = TRIM == "none"
# fp8 DoubleRow Q/K projections: x as hi+lo fp8 pair (quantization
# residual split), weights single fp8. V projection stays bf16.
PROJ_DR = os.environ.get("K_PROJ_DR", "") != ""
F8 = mybir.dt.float8e4


def _tp(pos):
    return pos if USE_TILE_POSITION else None


def build_program(t=T):
    nqb = t // QW
    nkb = t // 128

    nc = bacc.Bacc("TRN2", target_bir_lowering=False, debug=False,
                   num_devices=NCORES)

    xT_d = nc.dram_tensor("xT", [EMBED, t], BF16, kind="ExternalInput")
    wqkv_d = nc.dram_tensor("wqkvT", [EMBED, WCOLS], BF16,
                            kind="ExternalInput")
    if PROJ_DR:
        xhi_d = nc.dram_tensor("xhi", [EMBED, t], F8, kind="ExternalInput")
        xlo_d = nc.dram_tensor("xlo", [EMBED, t], F8, kind="ExternalInput")
        wdr_d = nc.dram_tensor("wdr", [EMBED, 512], F8, kind="ExternalInput")
    bqkv_d = nc.dram_tensor("bqkv", [WCOLS, 1], F32, kind="ExternalInput")
    wo_d = nc.dram_tensor("woT", [HPC * DH, EMBED], BF16,
                          kind="ExternalInput")
    y_d = nc.dram_tensor("y", [t, EMBED], BF16, kind="ExternalOutput")

    with tile.TileContext(nc) as tc:
        with (
            tc.tile_pool(name="const", bufs=1) as cpool,
            tc.tile_pool(name="persist", bufs=1) as perm,
        ):
            ident = cpool.tile([128, 128], BF16, tag="ident")
            make_identity(nc, ident)
            if not BF16_VT:
                identf = cpool.tile([128, 128], F32, tag="identf")
                make_identity(nc, identf)
            ones_t = cpool.tile([128, 64], F32R, tag="ones")
            nc.gpsimd.memset(ones_t.bitcast(F32), 1.0)

            wq_sb = [cpool.tile([128, WCOLS], BF16, name=f"wq{ct}",
                                tag=f"wq{ct}") for ct in range(6)]
            for ct in range(6):
                nc.sync.dma_start(wq_sb[ct],
                                  wqkv_d[ct * 128:(ct + 1) * 128, :])
            wo_sb = [cpool.tile([64, EMBED], BF16, name=f"wo{h}",
                                tag=f"wo{h}") for h in range(3)]
            for h in range(3):
                nc.sync.dma_start(wo_sb[h], wo_d[h * 64:(h + 1) * 64, :])
            bias_sb = []
            for mc in range(6):
                mw = 128 if mc < 5 else 64
                b_t = cpool.tile([128, 1], F32, name=f"bias{mc}",
                                 tag=f"bias{mc}")
                nc.sync.dma_start(b_t[:mw, :],
                                  bqkv_d[mc * 128:mc * 128 + mw, :])
                bias_sb.append(b_t)

            # resident xT tiles, DMA'd per-tb slices
            xt = [perm.tile([128, t], BF16, name=f"xt{ct}", tag=f"xt{ct}")
                  for ct in range(6)]
            if PROJ_DR:
                # [128, 2, t]-as-[128, 2t]: DR pair block j at cols [j*t:]
                xf8 = {}
                for ver in ("hi", "lo"):
                    xf8[ver] = [perm.tile([128, 2 * t], F8,
                                          name=f"x{ver}{ch}",
                                          tag=f"x{ver}{ch}")
                                for ch in range(3)]
                wdr_sb = [cpool.tile([128, 1024], F8, name=f"wdr{ch}",
                                     tag=f"wdr{ch}") for ch in range(3)]
                for ch in range(3):
                    nc.sync.dma_start(wdr_sb[ch][:, 0:512],
                                      wdr_d[ch * 256:ch * 256 + 128, :])
                    nc.sync.dma_start(wdr_sb[ch][:, 512:1024],
                                      wdr_d[ch * 256 + 128:ch * 256 + 256, :])
            q01 = perm.tile([128, t], BF16, tag="q01")
            k01 = perm.tile([128, t], BF16, tag="k01")
            q22 = perm.tile([128, t], BF16, tag="q22")
            k22 = perm.tile([128, t], BF16, tag="k22")
            vs = [perm.tile([128, nkb * 65], BF16, name=f"vs{h}",
                            tag=f"vs{h}") for h in range(3)]
            for h in range(3):
                nc.gpsimd.memset(vs[h], 1.0)

            qk_dest = [q01, k01, q22, k22]

            stack = contextlib.ExitStack()
            spsum = stack.enter_context(
                tc.tile_pool(name="spsum", bufs=2, space="PSUM"))
            accpsum = stack.enter_context(
                tc.tile_pool(name="accpsum", bufs=2, space="PSUM"))
            upsum = stack.enter_context(
                tc.tile_pool(name="upsum", bufs=2, space="PSUM"))
            ppool = stack.enter_context(tc.tile_pool(name="ppool", bufs=4))
            vtpool = stack.enter_context(tc.tile_pool(name="vtpool", bufs=2))
            rpool = stack.enter_context(tc.tile_pool(name="rpool", bufs=3))
            apool = stack.enter_context(tc.tile_pool(name="apool", bufs=2))
            ysb = stack.enter_context(tc.tile_pool(name="ysb", bufs=3))

            # ------------- projection extras (per PAIR of 512-row tbs) --
            # Two tbs are projected together so each weight chunk's
            # LDWEIGHTS serves two matmuls (consecutive same-lhsT matmuls
            # skip the reload).
            def a_chunks(tp):
                tbs = [tb for tb in (2 * tp, 2 * tp + 1) if tb < nqb]
                sls = [slice(tb * QW, (tb + 1) * QW) for tb in tbs]
                vtmp = {}

                def c_xload():
                    for ct in range(6):
                        for sl in sls:
                            nc.sync.dma_start(
                                xt[ct][:, sl],
                                xT_d[ct * 128:(ct + 1) * 128, sl])
                    if PROJ_DR:
                        for ch in range(3):
                            for ver, src in (("hi", xhi_d), ("lo", xlo_d)):
                                for j in range(2):
                                    r0 = ch * 256 + j * 128
                                    for sl in sls:
                                        dsl = slice(j * t + sl.start,
                                                    j * t + sl.stop)
                                        nc.sync.dma_start(
                                            xf8[ver][ch][:, dsl],
                                            src[r0:r0 + 128, sl])

                def c_proj(mc):
                    def f():
                        mw = 128 if mc < 5 else 64
                        c0 = mc * 128
                        pss = [upsum.tile([128, QW], F32, tag="u",
                                          name=f"proj{tb}_{mc}")
                               for tb in tbs]
                        if PROJ_DR and mc < 4:
                            wap = [wdr_sb[ch].rearrange(
                                "p (two m) -> p two m", two=2)[
                                :, :, c0:c0 + 128] for ch in range(3)]
                            for ch in range(3):
                                for vi, ver in enumerate(("hi", "lo")):
                                    xap = xf8[ver][ch].rearrange(
                                        "p (two c) -> p two c", two=2)
                                    for j in range(len(tbs)):
                                        nc.tensor.matmul(
                                            pss[j][:, :],
                                            lhsT=wap[ch],
                                            rhs=xap[:, :, sls[j]],
                                            perf_mode=(
                                                mybir.MatmulPerfMode
                                                .DoubleRow),
                                            start=(ch == 0 and vi == 0),
                                            stop=(ch == 2 and vi == 1))
                        else:
                            for ct in range(6):
                                for j in range(len(tbs)):
                                    nc.tensor.matmul(
                                        pss[j][:mw, :],
                                        lhsT=wq_sb[ct][:, c0:c0 + mw],
                                        rhs=xt[ct][:, sls[j]],
                                        start=(ct == 0), stop=(ct == 5))
                        for j in range(len(tbs)):
                            if mc < 4:
                                dest = qk_dest[mc][:, sls[j]]
                                nc.vector.tensor_scalar_add(
                                    dest, pss[j][:mw, :], bias_sb[mc][:mw, :])
                            else:
                                vt = vtpool.tile(
                                    [128, QW], BF16 if BF16_VT else F32,
                                    tag=f"vtmp{mc}",
                                    name=f"vtmp{tbs[j]}_{mc}")
                                vtmp[(mc, j)] = vt
                                nc.vector.tensor_scalar_add(
                                    vt[:mw, :], pss[j][:mw, :],
                                    bias_sb[mc][:mw, :])
                    return f

                def c_vt(h, j):
                    def f():
                        tb = tbs[j]
                        src = (vtmp[(4, j)][0:64], vtmp[(4, j)][64:128],
                               vtmp[(5, j)][0:64])[h]
                        idt = ident if BF16_VT else identf
                        idn = (idt[0:64, 0:64], idt[64:128, 64:128],
                               idt[0:64, 0:64])[h]
                        vtile = upsum.tile(
                            [128, 2 * QW] if BF16_VT else [128, QW],
                            BF16 if BF16_VT else F32, tag="u",
                            name=f"vt{h}_{tb}")
                        for i in range(4):
                            nc.tensor.transpose(
                                vtile[:, i * 64:(i + 1) * 64],
                                src[:, i * 128:(i + 1) * 128],
                                idn)
                        s2 = vtile[:, 0:256].rearrange(
                            "p (c w) -> p c w", w=64)
                        dst = vs[h].rearrange("p (c w) -> p c w", w=65)[
                            :, tb * 4:tb * 4 + 4, 0:64]
                        nc.vector.tensor_copy(dst, s2)
                    return f

                chunks = [c_xload]
                chunks += [c_proj(mc) for mc in range(6)]
                chunks += [c_vt(h, j) for j in range(len(tbs))
                           for h in range(3)]
                return chunks

            # ---------------- attention stages ---------------------------
            # group list: per qb, phase A (h0&h1 row-tiled pairs), then
            # phase B (h2 paired with itself via duplicated K/Q columns).
            groups = []
            for qb in range(nqb):
                ng = (qb + 1) * 2
                for g in range(ng):
                    groups.append((qb, "A", g, g == ng - 1))
                for g in range(ng):
                    groups.append((qb, "B", g, g == ng - 1))
            ngroups = len(groups)

            def rects(qb, g):
                """(kbi, in-tile col offset, width) for the kbi pair."""
                out = []
                rs = 0
                for kbi in (2 * g, 2 * g + 1):
                    off = (max(0, (kbi - 4 * qb) * 128)
                           if TRIM == "all" else 0)
                    w = QW - off
                    out.append((kbi, rs, off, w))
                    rs += w
                return out

            sp_t = {}
            pt_t = {}
            acc_t = {}
            rec_t = {}
            attn = {}
            deferred = {}

            def defer(slot, fn):
                deferred.setdefault(slot, []).append(fn)

            def emit_S(i):
                qb, ph, g, last = groups[i]
                q_base = qb * QW
                if ph == "A":
                    sA = spsum.tile([128, 2 * QW], F32, tag="s",
                                    name=f"sA{qb}_{g}")
                    sB = spsum.tile([128, 2 * QW], F32, tag="s",
                                    name=f"sB{qb}_{g}")
                    sp_t[i] = (sA, sB)
                    for kbi, rs, off, w in rects(qb, g):
                        k_sl = slice(kbi * 128, (kbi + 1) * 128)
                        q_sl = slice(q_base + off, q_base + QW)
                        nc.tensor.matmul(
                            sA[:, rs:rs + w],
                            lhsT=k01[0:64, k_sl], rhs=q01[0:64, q_sl],
                            start=True, stop=True, tile_position=_tp((0, 0)))
                        nc.tensor.matmul(
                            sB[:, rs:rs + w],
                            lhsT=k01[64:128, k_sl], rhs=q01[64:128, q_sl],
                            start=True, stop=True, tile_position=_tp((64, 0)))
                else:
                    sC = spsum.tile([128, 2 * QW], F32, tag="s",
                                    name=f"sC{qb}_{g}")
                    sp_t[i] = (sC,)
                    for idx, (kbi, rs, off, w) in enumerate(rects(qb, g)):
                        k_sl = slice(kbi * 128, (kbi + 1) * 128)
                        q_sl = slice(q_base + off, q_base + QW)
                        r0 = idx * 64
                        nc.tensor.matmul(
                            sC[:, rs:rs + w],
                            lhsT=k22[r0:r0 + 64, k_sl],
                            rhs=q22[r0:r0 + 64, q_sl],
                            start=True, stop=True, tile_position=_tp((r0, 0)))

            def emit_exp_mask(i):
                qb, ph, g, last = groups[i]
                rcs = rects(qb, g)
                wsum = sum(r[3] for r in rcs)
                hs = (0, 1) if ph == "A" else (2,)
                pts = []
                for hi, h in enumerate(hs):
                    sp = sp_t[i][hi]
                    pt = ppool.tile([128, 2 * QW], BF16, tag="p",
                                    name=f"p{qb}_{ph}{g}_{h}")
                    pts.append(pt)
                    nc.scalar.activation(pt[:, 0:wsum], sp[:, 0:wsum],
                                         Act.Exp, bias=0.0, scale=0.125)
                    for kbi, rs, off, w in rcs:
                        if kbi >= 4 * qb:
                            if TRIM == "none":
                                # PV reads the full rect: mask everything
                                # left of the diagonal too
                                v = pt[:, rs:rs + QW]
                                nc.gpsimd.affine_select(
                                    out=v, in_=v,
                                    compare_op=mybir.AluOpType.is_ge,
                                    fill=0.0, base=qb * QW - kbi * 128,
                                    pattern=[[1, QW]],
                                    channel_multiplier=-1)
                            else:
                                # PV starts reading at the diagonal block:
                                # only its 128x128 triangle needs masking
                                moff = 0 if TRIM == "all" else (
                                    (kbi - 4 * qb) * 128)
                                v = pt[:, rs + moff:rs + moff + 128]
                                nc.gpsimd.affine_select(
                                    out=v, in_=v,
                                    compare_op=mybir.AluOpType.is_ge,
                                    fill=0.0, base=0,
                                    pattern=[[1, 128]],
                                    channel_multiplier=-1)
                pt_t[i] = pts

            def emit_PV(i):
                qb, ph, g, last = groups[i]
                kbn = 4 * (qb + 1)
                hs = (0, 1) if ph == "A" else (2,)
                pts = pt_t.pop(i)
                for hi, h in enumerate(hs):
                    if g == 0:
                        acc_t[(qb, h)] = accpsum.tile(
                            [65, QW], F32, tag="acc", name=f"acc{qb}_{h}")
                    acc = acc_t[(qb, h)]
                    pt = pts[hi]
                    for kbi, rs, off, w in rects(qb, g):
                        if TRIM == "pv":
                            off = max(0, (kbi - 4 * qb) * 128)
                            rs = rs + off
                        nc.tensor.matmul(
                            acc[:, off:QW],
                            lhsT=vs[h][:, kbi * 65:kbi * 65 + 65],
                            rhs=pt[:, rs:rs + (QW - off)],
                            start=(kbi == 0), stop=(kbi == kbn - 1))
                sp_t.pop(i)

            def emit_recip(qb, h):
                acc = acc_t.pop((qb, h))
                accsb = rpool.tile([65, QW], F32, tag="accsb",
                                   name=f"accsb{qb}_{h}")
                nc.vector.tensor_copy(accsb, acc)
                rec = rpool.tile([65, QW], F32R, tag="rec",
                                 name=f"rec{qb}_{h}")
                rec_t[(qb, h)] = (accsb, rec)
                with nc.allow_low_precision(reason="fp32r rounding"):
                    nc.vector.reciprocal(rec[64:65], accsb[64:65])

            def emit_div(qb, h):
                accsb, rec = rec_t.pop((qb, h))
                bc = upsum.tile([128, QW], F32, tag="u",
                                name=f"bc{qb}_{h}")
                nc.tensor.matmul(bc[0:64, :], lhsT=ones_t[64:65, :],
                                 rhs=rec[64:65, :],
                                 start=True, stop=True)
                if h == 0:
                    attn[qb] = [apool.tile([64, QW], BF16, tag=f"attn{hh}",
                                           name=f"attn{hh}_{qb}")
                                for hh in range(3)]
                nc.vector.tensor_mul(attn[qb][h], accsb[0:64, :],
                                     bc[0:64, :])

            def emit_outproj(qb, mt):
                at = attn[qb]
                t_sl = slice(mt * 128, (mt + 1) * 128)
                row0 = qb * QW + mt * 128
                ys = ysb.tile([128, EMBED], BF16, tag="ys",
                              name=f"ys{qb}_{mt}")
                yps = [upsum.tile([128, QW], F32, tag="u",
                                  name=f"y{qb}_{mt}_{nh}")
                       for nh in range(2)]
                # h-major so both nh-halves reuse each at[h] weight load
                for h in range(3):
                    for nh in range(2):
                        n_sl = slice(nh * 384, (nh + 1) * 384)
                        nc.tensor.matmul(yps[nh][:, 0:384],
                                         lhsT=at[h][:, t_sl],
                                         rhs=wo_sb[h][:, n_sl],
                                         start=(h == 0), stop=(h == 2))
                for nh in range(2):
                    n_sl = slice(nh * 384, (nh + 1) * 384)
                    nc.vector.tensor_copy(ys[:, n_sl], yps[nh][:, 0:384])
                nc.sync.dma_start(y_d[row0:row0 + 128, :], ys)
                if mt == 3:
                    attn.pop(qb)

            # ---------------- main emission loop -------------------------
            # extras entries are (need_before_qb, fn): fn must have run
            # before groups of that qb start. Outproj chunks carry no
            # deadline (they only feed the y DMA).
            extras = []
            for fn in a_chunks(0):  # tb 0 and 1
                fn()

            emit_S(0)
            for i in range(ngroups):
                qb, ph, g, last = groups[i]
                if i + 1 < ngroups:
                    if groups[i + 1][0] != qb:
                        nxt = groups[i + 1][0]
                        # run ALL due chunks, not just a due prefix —
                        # deadline-free outproj chunks at the front must
                        # not block overdue projection chunks behind them
                        due = [fn for dl, fn in extras if dl <= nxt]
                        extras = [(dl, fn) for dl, fn in extras
                                  if dl > nxt]
                        for fn in due:
                            fn()
                        if qb % 2 == 0 and qb + 2 < nqb:
                            # pair (qb+2)//2 covers tb qb+2 (needed at
                            # qb+2) and tb qb+3 (needed at qb+3)
                            extras.extend(
                                (qb + 2, fn)
                                for fn in a_chunks((qb + 2) // 2))
                    emit_S(i + 1)
                emit_exp_mask(i)
                for fn in deferred.pop(i, ()):
                    fn()
                emit_PV(i)
                if last:
                    if ph == "A":
                        emit_recip(qb, 0)
                        emit_recip(qb, 1)
                        defer(i + 1, lambda qb=qb: emit_div(qb, 0))
                        defer(i + 1, lambda qb=qb: emit_div(qb, 1))
                    else:
                        emit_recip(qb, 2)
                        defer(i + 1, lambda qb=qb: emit_div(qb, 2))
                        defer(i + 1, lambda qb=qb: extras.extend(
                            (nqb + 1,
                             lambda qb=qb, mt=mt: emit_outproj(qb, mt))
                            for mt in range(4)))
                if extras:
                    extras.pop(0)[1]()
            for slot in sorted(deferred):
                for fn in deferred[slot]:
                    fn()
            while extras:
                extras.pop(0)[1]()
            stack.close()
    nc.compile()
    return nc


_PROG_CACHE = {}


def _get_program(t=T):
    if t not in _PROG_CACHE:
        _PROG_CACHE[t] = build_program(t)
    return _PROG_CACHE[t]


def _bf16(a):
    return np.ascontiguousarray(np.asarray(a, np.float32)).astype(
        ml_dtypes.bfloat16)


def make_in_maps(x, wq, bq, wk, bk, wv, bv, wo, t=T):
    in_maps = []
    for core in range(NCORES):
        b = core // 4
        hs = (core % 4) * HPC
        sl = [slice((hs + h) * DH, (hs + h + 1) * DH) for h in range(HPC)]
        wcols = [wq[sl[0]].T, wq[sl[1]].T,
                 wk[sl[0]].T, wk[sl[1]].T,
                 wq[sl[2]].T, wq[sl[2]].T,
                 wk[sl[2]].T, wk[sl[2]].T,
                 wv[sl[0]].T, wv[sl[1]].T,
                 wv[sl[2]].T]
        biases = [bq[sl[0]], bq[sl[1]], bk[sl[0]], bk[sl[1]],
                  bq[sl[2]], bq[sl[2]], bk[sl[2]], bk[sl[2]],
                  bv[sl[0]], bv[sl[1]], bv[sl[2]]]
        wqkvT = _bf16(np.concatenate(wcols, axis=1))
        bqkv = np.ascontiguousarray(
            np.concatenate(biases)[:, None], dtype=np.float32)
        ch = slice(hs * DH, (hs + HPC) * DH)
        woT = _bf16(wo[:, ch].T)
        xTf = np.ascontiguousarray(np.asarray(x[b][:t], np.float32).T)
        im = {
            "xT": xTf.astype(ml_dtypes.bfloat16),
            "wqkvT": wqkvT,
            "bqkv": bqkv,
            "woT": woT,
        }
        if PROJ_DR:
            xhi = xTf.astype(ml_dtypes.float8_e4m3)
            xlo = (xTf - xhi.astype(np.float32)).astype(ml_dtypes.float8_e4m3)
            wqk = np.concatenate(wcols[:8], axis=1).astype(np.float32)
            im["xhi"] = xhi
            im["xlo"] = xlo
            im["wdr"] = np.ascontiguousarray(wqk).astype(
                ml_dtypes.float8_e4m3)
        in_maps.append(im)
    return in_maps


def run(inputs, t=T, trace=False, **kw):
    arrs = {k: np.asarray(v, dtype=np.float32) for k, v in inputs.items()}
    nc = _get_program(t)
    in_maps = make_in_maps(**arrs, t=t)
    res = run_bass_kernel_spmd(nc, in_maps, list(range(NCORES)),
                               trace=trace, **kw)
    outs = [np.asarray(m["y"], dtype=np.float32) for m in res.results]
    y = np.empty((B, t, EMBED), dtype=np.float32)
    for b in range(B):
        y[b] = outs[4 * b] + outs[4 * b + 1] + outs[4 * b + 2] + outs[4 * b + 3]
    return y, res


def kernel(**inputs):
    y, _ = run(inputs)
    return y
